# revision 4
# baseline (speedup 1.0000x reference)
"""GRU (5-layer, H=128) Trainium2 Bass kernel, v4.

Data parallel over batch (64/8 = 8 per core).  Chunk-staggered wavefront,
chunk C=8: layer l processes chunk (m-l) in round m.  Single batched
per-step chain over all 5 layers (minimal dependency hops); PSUM gate
buffers double-buffered so bias/ih fills for round m+1 stream during
round m; x arrives host-transposed; xn gate pre-activations are copied
to SBUF once per round so the npre op is all-SBUF.

Per step j (all 5 layers batched, critical path = 7 links):
  PE  : r,z hh matmuls (10, first) then hn (5)
  ACT : rz = sigmoid(P[0:10])        [128,10,8]
  POOL: zh = z * h                   (off path)
  DVE : rn = r * P_hn
  DVE : npre = rn + xn_sb            (all-SBUF)
  ACT : nt = tanh(npre)
  DVE : ng = (z-1)*nt                (scalar_tensor_tensor)
  DVE : h' = zh - ng -> hW           (= z*h + (1-z)*n, sliced to active)

Inner loop is branch-free: inactive layers compute garbage which is
never written back (h' is sliced to the active layer range).

PSUM blocks (64 words = 8 steps x 8 batch), 20 used of 24:
  0-4 r(l) | 5-9 z(l) | 10-14 xn(l) | 15-19 hn(l)
Biases pre-accumulated by 3 bf16 ones-trick matmuls per buffer.
"""

import sys

for p in ("/opt/trn_rl_repo", "/opt/pypackages"):
    if p not in sys.path:
        sys.path.append(p)

import numpy as np
import ml_dtypes

BFNP = ml_dtypes.bfloat16

import concourse.bass as bass  # noqa: F401
import bass_rust
import concourse.mybir as mybir
import concourse.tile as tile
from concourse import bacc

F32 = mybir.dt.float32
BF16 = mybir.dt.bfloat16
AF = mybir.ActivationFunctionType
ALU = mybir.AluOpType

H = 128
L = 5
NCORE = 8
BC = 8
IN = 512
OUT = 96
C = 8  # chunk (timesteps per round)


def build_nc(T=512):
    NCH = T // C
    NR = NCH + L - 1
    WDIM = T + C * (L - 1) + 1

    nc = bacc.Bacc("TRN2", target_bir_lowering=False, debug=False)

    xp = nc.dram_tensor("xp", [H, 4, T, BC], BF16, kind="ExternalInput")
    whhT = nc.dram_tensor("whhT", [H, L, 3, H], BF16, kind="ExternalInput")
    wihT = nc.dram_tensor("wihT", [H, L - 1, 3, H], BF16, kind="ExternalInput")
    wih0T = nc.dram_tensor("wih0T", [H, 4, 3, H], BF16, kind="ExternalInput")
    fcT = nc.dram_tensor("fcT", [H, OUT], BF16, kind="ExternalInput")
    biasP8 = nc.dram_tensor("biasP8", [8, 3, H], BF16, kind="ExternalInput")
    ones8 = nc.dram_tensor("ones8", [8, 512], BF16, kind="ExternalInput")
    fcb = nc.dram_tensor("fcb", [BC, OUT], F32, kind="ExternalInput")
    y = nc.dram_tensor("y", [BC, OUT], F32, kind="ExternalOutput")

    with tile.TileContext(nc) as tc:
        with (
            tc.tile_pool(name="persist", bufs=1) as pp,
            tc.tile_pool(name="tmp", bufs=3) as tp,
            tc.tile_pool(name="pP", bufs=2, space="PSUM") as pP,
        ):
            hW = pp.tile([H, L, WDIM, BC], BF16, tag="hW")
            xT = pp.tile([H, 4, T, BC], BF16, tag="xT")
            whh_sb = pp.tile([H, L, 3, H], BF16, tag="whh")
            wih_sb = pp.tile([H, L - 1, 3, H], BF16, tag="wih")
            wih0_sb = pp.tile([H, 4, 3, H], BF16, tag="wih0")
            fcT_sb = pp.tile([H, OUT], BF16, tag="fcT")
            biasP_sb = pp.tile([8, 3, H], BF16, tag="biasP")
            ones_sb = pp.tile([8, 512], BF16, tag="ones8")
            fcb_sb = pp.tile([BC, OUT], F32, tag="fcb")

            nc.sync.dma_start(whh_sb[:, :, :, :], whhT[:, :, :, :])
            nc.sync.dma_start(wih_sb[:, :, :, :], wihT[:, :, :, :])
            nc.sync.dma_start(wih0_sb[:, :, :, :], wih0T[:, :, :, :])
            nc.sync.dma_start(fcT_sb[:, :], fcT[:, :])
            nc.sync.dma_start(biasP_sb[:, :, :], biasP8[:, :, :])
            nc.sync.dma_start(ones_sb[:, :], ones8[:, :])
            nc.sync.dma_start(fcb_sb[:, :], fcb[:, :])

            for l in range(L):
                nc.vector.memset(hW[:, l, C * l, :], 0.0)

            def load_x_chunk(m):
                nc.sync.dma_start(
                    xT[:, :, m * C : (m + 1) * C, :], xp[:, :, m * C : (m + 1) * C, :]
                )

            load_x_chunk(0)
            if NCH > 1:
                load_x_chunk(1)

            def active(m):
                la0 = max(0, m - (NCH - 1))
                la1 = min(L - 1, m)
                return la0, la1

            def relax(consumer, producer):
                """Same-engine adjacent dep: drop the semaphore wait, keep
                program order (in-order engine + post-op drain make the
                write visible to the next instruction)."""
                ci, pi = consumer.ins, producer.ins
                try:
                    ci.remove_dependency(pi.name)
                    ci.add_dependency(
                        pi.name, bass_rust.DependencyInfo(sync=False, no_sync=True)
                    )
                except Exception:
                    pass

            def mm(dest, lhsT, rhs, start=False, stop=False):
                nc.tensor.matmul(
                    dest, lhsT, rhs, start=start, stop=stop, skip_group_check=True
                )

            def bias_fills(Pn):
                f = Pn[:, :, :, :].rearrange("p a c b -> p (a c b)")
                mm(f[:, 0:512], biasP_sb[0:8, 0, :], ones_sb[0:8, 0:512], start=True)
                mm(f[:, 512:1024], biasP_sb[0:8, 1, :], ones_sb[0:8, 0:512], start=True)
                mm(f[:, 1024:1280], biasP_sb[0:4, 2, :], ones_sb[0:4, 0:256], start=True)

            def ih_fills_l0(Pn, m1):
                # layer 0 r/z/xn from xT chunk m1 (no chain dependency)
                for g, blk in ((0, 0), (1, 5), (2, 10)):
                    for ki in range(4):
                        mm(
                            Pn[:, blk, :, :],
                            wih0_sb[:, ki, g, :],
                            xT[:, ki, m1 * C : (m1 + 1) * C, :],
                        )

            def ih_fills_rest(Pn, m1):
                la0, la1 = active(m1)
                for l in range(max(1, la0), la1 + 1):
                    rhs = hW[:, l - 1, C * m1 - C + 1 : C * m1 + 1, :]
                    for g, blk in ((0, l), (1, 5 + l), (2, 10 + l)):
                        mm(Pn[:, blk, :, :], wih_sb[:, l - 1, g, :], rhs)

            # ---- prologue: fill round 0 ----
            P_cur = pP.tile([H, 24, C, BC], F32, tag="P")
            bias_fills(P_cur)
            ih_fills_l0(P_cur, 0)

            for m in range(NR):
                la0, la1 = active(m)

                if m + 2 < NCH:
                    load_x_chunk(m + 2)

                P_next = None
                if m + 1 < NR:
                    P_next = pP.tile([H, 24, C, BC], F32, tag="P")
                    bias_fills(P_next)
                    if m + 1 < NCH:
                        ih_fills_l0(P_next, m + 1)

                # xn gate pre-activations for this round -> SBUF (one ACT copy)
                xnc = tp.tile([H, L, C, BC], F32, tag="xnc")
                nc.vector.tensor_scalar(
                    xnc[:, :, :, :], P_cur[:, 10:15, :, :], 0.0, None, ALU.add
                )

                for j in range(C):
                    base = C * m + j
                    # ---- PE: r matmuls first (sig_r waits only these 5),
                    # then hn (rn input), then z (sig_z is off-path) ----
                    for blk0, g in ((0, 0), (15, 2), (5, 1)):
                        for l in range(L):
                            mm(
                                P_cur[:, blk0 + l, j, :],
                                whh_sb[:, l, g, :],
                                hW[:, l, base, :],
                                stop=True,
                            )

                    rz = tp.tile([H, 10, BC], F32, tag="rz")
                    rn = tp.tile([H, L, BC], F32, tag="rn")
                    npre = tp.tile([H, L, BC], F32, tag="npre")
                    nt = tp.tile([H, L, BC], F32, tag="nt")
                    zh = tp.tile([H, L, BC], F32, tag="zh")
                    ng = tp.tile([H, L, BC], F32, tag="ng")

                    nc.scalar.activation(
                        rz[:, 0:5, :], P_cur[:, 0:5, j, :], AF.Sigmoid
                    )
                    nc.vector.tensor_tensor(
                        rn[:, :, :], rz[:, 0:5, :], P_cur[:, 15:20, j, :], ALU.mult
                    )
                    nc.scalar.activation(
                        rz[:, 5:10, :], P_cur[:, 5:10, j, :], AF.Sigmoid
                    )
                    nc.vector.tensor_tensor(
                        npre[:, :, :], rn[:, :, :], xnc[:, :, j, :], ALU.add
                    )
                    nc.gpsimd.tensor_tensor(
                        zh[:, :, :], rz[:, 5:10, :], hW[:, :, base, :], ALU.mult
                    )
                    nc.scalar.activation(nt[:, :, :], npre[:, :, :], AF.Tanh)
                    # ng = (z-1)*nt ; h' = zh - ng = z*h + (1-z)*n
                    nc.vector.scalar_tensor_tensor(
                        ng[:, :, :], rz[:, 5:10, :], 1.0, nt[:, :, :],
                        ALU.subtract, ALU.mult,
                    )
                    nc.vector.tensor_tensor(
                        hW[:, la0 : la1 + 1, base + 1, :],
                        zh[:, la0 : la1 + 1, :],
                        ng[:, la0 : la1 + 1, :],
                        ALU.subtract,
                    )


                if P_next is not None:
                    ih_fills_rest(P_next, m + 1)
                P_cur = P_next

            # ---- final FC on h_4(T) ----
            with tc.tile_pool(name="pfc", bufs=1, space="PSUM") as pfcp:
                pfc = pfcp.tile([BC, OUT], F32, tag="fc")
                mm(pfc[:, :], hW[:, L - 1, WDIM - 1, :], fcT_sb[:, :], start=True, stop=True)
                out_sb = pp.tile([BC, OUT], F32, tag="out")
                nc.vector.tensor_tensor(out_sb[:, :], pfc[:, :], fcb_sb[:, :], ALU.add)
                nc.sync.dma_start(y[:, :], out_sb[:, :])

    nc.compile()
    return nc


def prep_shared(w_ih0, w_ih_rest, w_hh, b_ih, b_hh, fc_w, fc_b):
    d = {}
    whhT = np.empty([H, L, 3, H], np.float32)
    for l in range(L):
        for g in range(3):
            whhT[:, l, g, :] = w_hh[l, g * H : (g + 1) * H, :].T
    d["whhT"] = whhT.astype(BFNP)
    wihT = np.empty([H, L - 1, 3, H], np.float32)
    for l in range(1, L):
        for g in range(3):
            wihT[:, l - 1, g, :] = w_ih_rest[l - 1, g * H : (g + 1) * H, :].T
    d["wihT"] = wihT.astype(BFNP)
    wih0T = np.empty([H, 4, 3, H], np.float32)
    for ki in range(4):
        for g in range(3):
            wih0T[:, ki, g, :] = w_ih0[g * H : (g + 1) * H, ki * H : (ki + 1) * H].T
    d["wih0T"] = wih0T.astype(BFNP)
    d["fcT"] = np.ascontiguousarray(fc_w.T).astype(BFNP)

    br = [b_ih[l, 0:H] + b_hh[l, 0:H] for l in range(L)]
    bz = [b_ih[l, H : 2 * H] + b_hh[l, H : 2 * H] for l in range(L)]
    bxn = [b_ih[l, 2 * H : 3 * H] for l in range(L)]
    bhn = [b_hh[l, 2 * H : 3 * H] for l in range(L)]

    def bias_of_block(blk):
        # 0-4 r | 5-9 z | 10-14 xn | 15-19 hn
        g, l = divmod(blk, 5)
        return (br, bz, bxn, bhn)[g][l]

    biasP8 = np.zeros([8, 3, H], np.float32)
    for blk in range(20):
        s, k = divmod(blk, 8)
        biasP8[k, s, :] = bias_of_block(blk)
    d["biasP8"] = biasP8.astype(BFNP)
    ones8 = np.zeros([8, 512], np.float32)
    for k in range(8):
        ones8[k, k * 64 : (k + 1) * 64] = 1.0
    d["ones8"] = ones8.astype(BFNP)
    d["fcb"] = np.tile(fc_b.astype(np.float32)[None, :], (BC, 1))
    return d


_NC_CACHE = {}


def run(x, w_ih0, w_ih_rest, w_hh, b_ih, b_hh, fc_w, fc_b, T=512, **run_kwargs):
    from concourse.bass_utils import run_bass_kernel_spmd

    if T not in _NC_CACHE:
        _NC_CACHE[T] = build_nc(T)
    nc = _NC_CACHE[T]
    shared = prep_shared(
        np.asarray(w_ih0), np.asarray(w_ih_rest), np.asarray(w_hh),
        np.asarray(b_ih), np.asarray(b_hh), np.asarray(fc_w), np.asarray(fc_b),
    )
    x = np.asarray(x)
    in_maps = []
    for c in range(NCORE):
        m = dict(shared)
        xs = x[c * BC : (c + 1) * BC, :T, :]  # [BC, T, IN]
        xt = np.ascontiguousarray(
            xs.transpose(2, 1, 0).reshape(4, H, T, BC).transpose(1, 0, 2, 3)
        )
        m["xp"] = xt.astype(BFNP)
        in_maps.append(m)
    res = run_bass_kernel_spmd(nc, in_maps, core_ids=list(range(NCORE)), **run_kwargs)
    out = np.concatenate([res.results[c]["y"] for c in range(NCORE)], axis=0)
    return out, res


def kernel(x, w_ih0, w_ih_rest, w_hh, b_ih, b_hh, fc_w, fc_b):
    out, _ = run(x, w_ih0, w_ih_rest, w_hh, b_ih, b_hh, fc_w, fc_b, T=512)
    return out.astype(np.float32)


if __name__ == "__main__":
    T = int(sys.argv[1]) if len(sys.argv) > 1 else 64
    rng = np.random.default_rng(0)
    s = 1.0 / np.sqrt(H)
    u = lambda *sh: rng.uniform(-s, s, sh).astype(np.float32)
    x = rng.standard_normal((64, T, IN), dtype=np.float32)
    w_ih0 = u(3 * H, IN)
    w_ih_rest = u(L - 1, 3 * H, H)
    w_hh = u(L, 3 * H, H)
    b_ih = u(L, 3 * H)
    b_hh = u(L, 3 * H)
    fc_w = u(OUT, H)
    fc_b = u(OUT)

    def np_ref():
        sig = lambda v: 1.0 / (1.0 + np.exp(-v))
        h_in = x.astype(np.float64)
        for l in range(L):
            wi = (w_ih0 if l == 0 else w_ih_rest[l - 1]).astype(np.float64)
            wh = w_hh[l].astype(np.float64)
            gx = np.einsum("bti,gi->btg", h_in, wi) + b_ih[l]
            h = np.zeros((64, H))
            hs = []
            for t in range(T):
                gh = h @ wh.T + b_hh[l]
                xr, xz, xn = np.split(gx[:, t], 3, -1)
                hr, hz, hn = np.split(gh, 3, -1)
                r = sig(xr + hr)
                z = sig(xz + hz)
                n = np.tanh(xn + r * hn)
                h = (1 - z) * n + z * h
                hs.append(h)
            h_in = np.stack(hs, 1)
        return h_in[:, -1] @ fc_w.astype(np.float64).T + fc_b

    exp = np_ref()
    got, res = run(x, w_ih0, w_ih_rest, w_hh, b_ih, b_hh, fc_w, fc_b, T=T)
    err = np.abs(got - exp)
    rel = np.linalg.norm(got - exp) / np.linalg.norm(exp)
    print("max abs err:", err.max(), "rel:", rel)
    print("exec_time_ns:", res.exec_time_ns)
